# revision 13
# baseline (speedup 1.0000x reference)
"""Trainium2 Bass kernel: EventSceneGraph GNN message passing.

Reference semantics (per scene b):
  1. top-KA of actor_valid-masked spike_rate -> aidx
  2. lane_dist[l] = min_j dist(x_centers[b, aidx[j]], lane_centers[b, l]);
     top-KL smallest over lane_valid lanes -> lidx
  3. nodes = concat(actor_feat[b, aidx], lane_feat[b, lidx])       [KA+KL, D]
  4. two residual MLP+LayerNorm blocks (rowwise over nodes)
  5. outputs = feature pools with selected rows replaced

Sharding: pure data parallel over the scene axis B=512 -> 64 scenes on each
of the 8 NeuronCores; MLP/LN params replicated.  No collectives.
"""

import os

import numpy as np

import concourse.bacc as bacc
import concourse.mybir as mybir
import concourse.tile as tile
from concourse.bass import IndirectOffsetOnAxis
from concourse.bass_utils import run_bass_kernel_spmd
from concourse.masks import make_identity

B, NA, NL, D = 512, 128, 512, 128
KA, KL = 16, 16
N_CORES = 8
BL = B // N_CORES  # scenes per core
BIG = 1e30
EPS = 1e-5

F32 = mybir.dt.float32
I32 = mybir.dt.int32
U32 = mybir.dt.uint32
AF = mybir.ActivationFunctionType
OP = mybir.AluOpType

# Abramowitz & Stegun 7.1.26 erf approximation (|err| <= 1.5e-7)
_AS_P = 0.3275911
_AS_A = (0.254829592, -0.284496736, 1.421413741, -1.453152027, 1.061405429)
_INV_SQRT2 = 0.7071067811865476

LAST_EXEC_NS = None


def _emit_gelu(nc, pool, out_sb, in_psum, mode):
    """out_sb = gelu(in_psum), exact (erf) flavor."""
    if mode == "hw":
        nc.scalar.activation(out_sb[:], in_psum[:], AF.Gelu)
        return
    # erf-composed gelu: gelu(x) = 0.5*x + 0.5*|x|*erf(|x|/sqrt(2))
    sh = list(out_sb.shape)
    ax = pool.tile(sh, F32, tag="g_ax")
    nc.scalar.activation(ax[:], in_psum[:], AF.Abs)
    z2 = pool.tile(sh, F32, tag="g_z2")
    nc.scalar.activation(z2[:], in_psum[:], AF.Square, scale=_INV_SQRT2)
    e = pool.tile(sh, F32, tag="g_e")
    nc.scalar.activation(e[:], z2[:], AF.Exp, scale=-1.0)
    u = pool.tile(sh, F32, tag="g_u")
    nc.vector.tensor_scalar(u[:], ax[:], _AS_P * _INV_SQRT2, 1.0, op0=OP.mult, op1=OP.add)
    t = pool.tile(sh, F32, tag="g_t")
    nc.vector.reciprocal(t[:], u[:])
    a1, a2, a3, a4, a5 = _AS_A
    h = pool.tile(sh, F32, tag="g_h")
    nc.vector.tensor_scalar(h[:], t[:], a5, a4, op0=OP.mult, op1=OP.add)
    for c in (a3, a2, a1):
        hm = pool.tile(sh, F32, tag="g_hm")
        nc.vector.tensor_tensor(hm[:], h[:], t[:], op=OP.mult)
        h = pool.tile(sh, F32, tag="g_h")
        nc.vector.tensor_scalar(h[:], hm[:], 1.0, c, op0=OP.mult, op1=OP.add)
    poly_t = pool.tile(sh, F32, tag="g_pt")
    nc.vector.tensor_tensor(poly_t[:], h[:], t[:], op=OP.mult)
    pe = pool.tile(sh, F32, tag="g_pe")
    nc.vector.tensor_tensor(pe[:], poly_t[:], e[:], op=OP.mult)
    erf_t = pool.tile(sh, F32, tag="g_erf")
    nc.vector.tensor_scalar(erf_t[:], pe[:], -1.0, 1.0, op0=OP.mult, op1=OP.add)
    xe = pool.tile(sh, F32, tag="g_xe")
    nc.vector.tensor_tensor(xe[:], ax[:], erf_t[:], op=OP.mult)
    halfx = pool.tile(sh, F32, tag="g_hx")
    nc.scalar.mul(halfx[:], in_psum[:], 0.5)
    xeh = pool.tile(sh, F32, tag="g_xeh")
    nc.vector.tensor_scalar(xeh[:], xe[:], 0.5, None, op0=OP.mult)
    nc.vector.tensor_tensor(out_sb[:], halfx[:], xeh[:], op=OP.add)


def _build_program(gelu_mode="hw", stage="full"):
    nc = bacc.Bacc("TRN2", target_bir_lowering=False, debug=False)

    af = nc.dram_tensor("actor_feat", [BL * NA, D], F32, kind="ExternalInput")
    lf = nc.dram_tensor("lane_feat", [BL * NL, D], F32, kind="ExternalInput")
    lc = nc.dram_tensor("lane_centers", [BL, NL, 2], F32, kind="ExternalInput")
    xc = nc.dram_tensor("x_centers", [BL * NA, 2], F32, kind="ExternalInput")
    sr = nc.dram_tensor("spike_rate", [BL, NA], F32, kind="ExternalInput")
    av = nc.dram_tensor("actor_valid", [BL, NA], F32, kind="ExternalInput")
    lv = nc.dram_tensor("lane_valid", [BL, NL], F32, kind="ExternalInput")
    wts = {
        n: nc.dram_tensor(n, [D, D], F32, kind="ExternalInput")
        for n in ("w0a", "w0b", "w1a", "w1b")
    }
    bias = {
        n: nc.dram_tensor(n, [1, D], F32, kind="ExternalInput")
        for n in ("b0a", "b0b", "b1a", "b1b")
    }
    gam = nc.dram_tensor("gamma_rep", [D, D], F32, kind="ExternalInput")
    bet = nc.dram_tensor("beta_rep", [D, D], F32, kind="ExternalInput")

    ao = nc.dram_tensor("actor_out", [BL * NA, D], F32, kind="ExternalOutput")
    lo = nc.dram_tensor("lane_out", [BL * NL, D], F32, kind="ExternalOutput")

    do_select = stage in ("select", "passthrough", "full")
    do_mlp = stage == "full"
    do_scatter = stage in ("passthrough", "full")

    with tile.TileContext(nc) as tc:
        with (
            tc.tile_pool(name="const", bufs=1) as cp,
            tc.tile_pool(name="work", bufs=3) as wp,
            tc.tile_pool(name="mlp", bufs=3) as mp,
            tc.tile_pool(name="stats", bufs=4) as sp,
            tc.tile_pool(name="psum", bufs=2, space="PSUM") as pp,
        ):
            # ---- small input loads + constants ----
            spike_sb = cp.tile([BL, NA], F32, tag="spike")
            nc.sync.dma_start(spike_sb[:], sr[:])
            av_sb = cp.tile([BL, NA], F32, tag="av")
            nc.sync.dma_start(av_sb[:], av[:])
            lv_sb = cp.tile([BL, NL], F32, tag="lv")
            nc.sync.dma_start(lv_sb[:], lv[:])
            lc_sb = cp.tile([BL, NL, 2], F32, tag="lc")
            nc.sync.dma_start(lc_sb[:], lc[:])
            w_sb = {}
            for n, t in wts.items():
                w_sb[n] = cp.tile([D, D], F32, tag=n, name=f"{n}_sb")
                nc.sync.dma_start(w_sb[n][:], t[:])
            b_sb = {}
            for n, t in bias.items():
                b_sb[n] = cp.tile([1, D], F32, tag=n, name=f"{n}_sb")
                nc.sync.dma_start(b_sb[n][:], t[:])
            gam_sb = cp.tile([D, D], F32, tag="gam")
            nc.sync.dma_start(gam_sb[:], gam[:])
            bet_sb = cp.tile([D, D], F32, tag="bet")
            nc.sync.dma_start(bet_sb[:], bet[:])

            ident = cp.tile([D, D], F32, tag="ident")
            make_identity(nc, ident[:])
            ones1 = cp.tile([1, D], F32, tag="ones1")
            nc.vector.memset(ones1[:], 1.0)
            base128 = cp.tile([BL, KA], I32, tag="base128")
            nc.gpsimd.iota(base128[:], pattern=[[0, KA]], base=0, channel_multiplier=NA)
            base512 = cp.tile([BL, KL], I32, tag="base512")
            nc.gpsimd.iota(base512[:], pattern=[[0, KL]], base=0, channel_multiplier=NL)

            # ---- bulk passthrough copies (DRAM->DRAM), overlap everything ----
            copy_insts_a = []
            for i in range(0, BL * NA, 4096):
                copy_insts_a.append(
                    nc.sync.dma_start(ao[i : i + 4096, :], af[i : i + 4096, :])
                )
            copy_insts_l = []
            for i in range(0, BL * NL, 4096):
                copy_insts_l.append(
                    nc.sync.dma_start(lo[i : i + 4096, :], lf[i : i + 4096, :])
                )

            afidx = None
            lfidx = None
            if do_select:
                # ---- actor top-KA over masked spike rate ----
                masked0 = wp.tile([BL, NA], F32, tag="amask0")
                nc.vector.tensor_scalar(masked0[:], av_sb[:], BIG, -BIG, op0=OP.mult, op1=OP.add)
                masked = wp.tile([BL, NA], F32, tag="amask")
                nc.vector.tensor_tensor(masked[:], masked0[:], spike_sb[:], op=OP.add)
                aidx = cp.tile([BL, KA], U32, tag="aidx")
                vals8 = wp.tile([BL, 8], F32, tag="avals8")
                nc.vector.max(vals8[:], masked[:])
                nc.vector.max_index(aidx[:, 0:8], vals8[:], masked[:])
                masked2 = wp.tile([BL, NA], F32, tag="amask2")
                nc.vector.match_replace(masked2[:], vals8[:], masked[:], -BIG)
                vals8b = wp.tile([BL, 8], F32, tag="avals8b")
                nc.vector.max(vals8b[:], masked2[:])
                nc.vector.max_index(aidx[:, 8:16], vals8b[:], masked2[:])
                aidx_i = cp.tile([BL, KA], I32, tag="aidx_i")
                nc.vector.tensor_copy(aidx_i[:], aidx[:])
                afidx = cp.tile([BL, KA], I32, tag="afidx")
                nc.vector.tensor_tensor(afidx[:], aidx_i[:], base128[:], op=OP.add)

                # ---- gather active-agent centers ----
                # HW indirect DMA consumes ONE offset per dest partition, so
                # gather slot-by-slot with [BL,1] offsets.
                ac = cp.tile([BL, KA, 2], F32, tag="ac")
                for j in range(KA):
                    nc.gpsimd.indirect_dma_start(
                        out=ac[:, j, :],
                        out_offset=None,
                        in_=xc[:],
                        in_offset=IndirectOffsetOnAxis(ap=afidx[:, j : j + 1], axis=0),
                    )

                # ---- per-lane min squared distance over the KA agents ----
                dmin = cp.tile([BL, NL], F32, tag="dmin")
                nc.vector.memset(dmin[:], BIG)
                for j in range(KA):
                    dx = wp.tile([BL, NL], F32, tag="dx")
                    nc.vector.tensor_scalar(dx[:], lc_sb[:, :, 0], ac[:, j, 0:1], None, op0=OP.subtract)
                    dy = wp.tile([BL, NL], F32, tag="dy")
                    nc.vector.tensor_scalar(dy[:], lc_sb[:, :, 1], ac[:, j, 1:2], None, op0=OP.subtract)
                    dx2 = wp.tile([BL, NL], F32, tag="dx2")
                    nc.scalar.square(dx2[:], dx[:])
                    dy2 = wp.tile([BL, NL], F32, tag="dy2")
                    nc.scalar.square(dy2[:], dy[:])
                    ss = wp.tile([BL, NL], F32, tag="ss")
                    nc.gpsimd.tensor_tensor(ss[:], dx2[:], dy2[:], op=OP.add)
                    nc.vector.tensor_tensor(dmin[:], dmin[:], ss[:], op=OP.min)

                # ---- lane top-KL (largest negated masked distance) ----
                pen = wp.tile([BL, NL], F32, tag="pen")
                nc.vector.tensor_scalar(pen[:], lv_sb[:], BIG, -BIG, op0=OP.mult, op1=OP.add)
                nld = wp.tile([BL, NL], F32, tag="nld")
                nc.vector.tensor_tensor(nld[:], pen[:], dmin[:], op=OP.subtract)
                lidx = cp.tile([BL, KL], U32, tag="lidx")
                lvals8 = wp.tile([BL, 8], F32, tag="lvals8")
                nc.vector.max(lvals8[:], nld[:])
                nc.vector.max_index(lidx[:, 0:8], lvals8[:], nld[:])
                nld2 = wp.tile([BL, NL], F32, tag="nld2")
                nc.vector.match_replace(nld2[:], lvals8[:], nld[:], -BIG)
                lvals8b = wp.tile([BL, 8], F32, tag="lvals8b")
                nc.vector.max(lvals8b[:], nld2[:])
                nc.vector.max_index(lidx[:, 8:16], lvals8b[:], nld2[:])
                lidx_i = cp.tile([BL, KL], I32, tag="lidx_i")
                nc.vector.tensor_copy(lidx_i[:], lidx[:])
                lfidx = cp.tile([BL, KL], I32, tag="lfidx")
                nc.vector.tensor_tensor(lfidx[:], lidx_i[:], base512[:], op=OP.add)

                # ---- offset re-layout [BL,K] -> [128, BL/8] (one offset per
                # dest partition per batch of 128 rows = 8 scenes x 16 slots)
                with tc.tile_pool(name="dram", bufs=1, space="DRAM") as dp:
                    stg_a = dp.tile([BL, KA], I32, tag="stg_a")
                    nc.sync.dma_start(stg_a[:], afidx[:])
                    ofs_a = cp.tile([128, BL // 8], I32, tag="ofs_a")
                    nc.sync.dma_start(
                        ofs_a[:], stg_a[:].rearrange("(g sg) j -> sg j g", g=BL // 8)
                    )
                    stg_l = dp.tile([BL, KL], I32, tag="stg_l")
                    nc.sync.dma_start(stg_l[:], lfidx[:])
                    ofs_l = cp.tile([128, BL // 8], I32, tag="ofs_l")
                    nc.sync.dma_start(
                        ofs_l[:], stg_l[:].rearrange("(g sg) j -> sg j g", g=BL // 8)
                    )

            if do_select:
                # ---- per-batch gather -> MLP -> scatter ----
                layers = [
                    (w_sb["w0a"], b_sb["b0a"], w_sb["w0b"], b_sb["b0b"]),
                    (w_sb["w1a"], b_sb["b1a"], w_sb["w1b"], b_sb["b1b"]),
                ]
                n_grp = BL // 8
                for ci in range(2 * n_grp):
                    if ci < n_grp:
                        feat, outt, ofs, g, cdeps = af, ao, ofs_a, ci, copy_insts_a
                    else:
                        feat, outt, ofs, g, cdeps = lf, lo, ofs_l, ci - n_grp, copy_insts_l
                    x = mp.tile([D, D], F32, tag="x0", name="x_in")
                    nc.gpsimd.indirect_dma_start(
                        out=x[:],
                        out_offset=None,
                        in_=feat[:],
                        in_offset=IndirectOffsetOnAxis(ap=ofs[:, g : g + 1], axis=0),
                    )
                    for (wa, ba, wb, bb) in (layers if do_mlp else []):
                        tp = pp.tile([D, D], F32, tag="tp")
                        nc.tensor.transpose(tp[:], x[:], ident[:])
                        xT = mp.tile([D, D], F32, tag="xT")
                        nc.scalar.copy(xT[:], tp[:])
                        hp = pp.tile([D, D], F32, tag="hp")
                        nc.tensor.matmul(hp[:], lhsT=xT[:], rhs=wa[:], start=True, stop=False)
                        nc.tensor.matmul(hp[:], lhsT=ones1[:], rhs=ba[:], start=False, stop=True)
                        gact = mp.tile([D, D], F32, tag="gact")
                        _emit_gelu(nc, mp, gact, hp, gelu_mode)
                        tp2 = pp.tile([D, D], F32, tag="tp2")
                        nc.tensor.transpose(tp2[:], gact[:], ident[:])
                        gT = mp.tile([D, D], F32, tag="gT")
                        nc.scalar.copy(gT[:], tp2[:])
                        h2 = pp.tile([D, D], F32, tag="h2")
                        nc.tensor.matmul(h2[:], lhsT=gT[:], rhs=wb[:], start=True, stop=False)
                        nc.tensor.matmul(h2[:], lhsT=ones1[:], rhs=bb[:], start=False, stop=True)

                        # residual + LayerNorm (rowwise over D)
                        # (InstTensorTensorReduce crashes this runtime's exec
                        # unit -> use tensor_tensor + tensor_reduce + ACT
                        # Square-with-accum instead)
                        x1 = mp.tile([D, D], F32, tag="x1")
                        nc.vector.tensor_tensor(x1[:], x[:], h2[:], op=OP.add)
                        sums = sp.tile([D, 1], F32, tag="sums")
                        nc.vector.tensor_reduce(sums[:], x1[:], axis=mybir.AxisListType.X, op=OP.add)
                        sq = mp.tile([D, D], F32, tag="sq")
                        sumsq = sp.tile([D, 1], F32, tag="sumsq")
                        nc.scalar.activation(sq[:], x1[:], AF.Square, accum_out=sumsq[:])
                        negm = sp.tile([D, 1], F32, tag="negm")
                        nc.vector.tensor_scalar(negm[:], sums[:], -1.0 / D, None, op0=OP.mult)
                        msq = sp.tile([D, 1], F32, tag="msq")
                        nc.vector.tensor_tensor(msq[:], negm[:], negm[:], op=OP.mult)
                        v1 = sp.tile([D, 1], F32, tag="v1")
                        nc.vector.tensor_scalar(v1[:], sumsq[:], 1.0 / D, EPS, op0=OP.mult, op1=OP.add)
                        v2 = sp.tile([D, 1], F32, tag="v2")
                        nc.vector.tensor_tensor(v2[:], v1[:], msq[:], op=OP.subtract)
                        sd = sp.tile([D, 1], F32, tag="sd")
                        nc.scalar.sqrt(sd[:], v2[:])
                        rstd = sp.tile([D, 1], F32, tag="rstd")
                        nc.vector.reciprocal(rstd[:], sd[:])
                        negmr = sp.tile([D, 1], F32, tag="negmr")
                        nc.vector.tensor_tensor(negmr[:], negm[:], rstd[:], op=OP.mult)
                        xh = mp.tile([D, D], F32, tag="xh")
                        nc.vector.tensor_scalar(xh[:], x1[:], rstd[:, 0:1], negmr[:, 0:1], op0=OP.mult, op1=OP.add)
                        xg = mp.tile([D, D], F32, tag="xg")
                        nc.gpsimd.tensor_tensor(xg[:], xh[:], gam_sb[:], op=OP.mult)
                        xn = mp.tile([D, D], F32, tag="x0", name="x_out")
                        nc.gpsimd.tensor_tensor(xn[:], xg[:], bet_sb[:], op=OP.add)
                        x = xn
                    if do_scatter:
                        sc = nc.gpsimd.indirect_dma_start(
                            out=outt[:],
                            out_offset=IndirectOffsetOnAxis(ap=ofs[:, g : g + 1], axis=0),
                            in_=x[:],
                            in_offset=None,
                        )
                        for c in cdeps:
                            tile.add_dep_helper(sc.ins, c.ins, reason="scatter after bulk copy")

    nc.compile()
    return nc


_PROGRAMS = {}


def _get_program(gelu_mode, stage="full"):
    key = (gelu_mode, stage)
    if key not in _PROGRAMS:
        _PROGRAMS[key] = _build_program(gelu_mode, stage)
    return _PROGRAMS[key]


def shard_inputs(actor_feat, lane_feat, lane_centers, x_centers, spike_rate,
                 actor_valid, lane_valid,
                 W0a, b0a, W0b, b0b, W1a, b1a, W1b, b1b, gamma, beta):
    f32 = lambda a: np.ascontiguousarray(np.asarray(a), dtype=np.float32)
    actor_feat = f32(actor_feat)
    lane_feat = f32(lane_feat)
    lane_centers = f32(lane_centers)
    x_centers = f32(x_centers)
    spike_rate = f32(spike_rate)
    avalid = np.asarray(actor_valid).astype(np.float32)
    lvalid = np.asarray(lane_valid).astype(np.float32)
    shared = {
        "w0a": f32(W0a), "w0b": f32(W0b), "w1a": f32(W1a), "w1b": f32(W1b),
        "b0a": f32(b0a).reshape(1, D), "b0b": f32(b0b).reshape(1, D),
        "b1a": f32(b1a).reshape(1, D), "b1b": f32(b1b).reshape(1, D),
        "gamma_rep": np.ascontiguousarray(np.tile(f32(gamma).reshape(1, D), (D, 1))),
        "beta_rep": np.ascontiguousarray(np.tile(f32(beta).reshape(1, D), (D, 1))),
    }
    in_maps = []
    for c in range(N_CORES):
        sl = slice(c * BL, (c + 1) * BL)
        m = {
            "actor_feat": np.ascontiguousarray(actor_feat[sl].reshape(BL * NA, D)),
            "lane_feat": np.ascontiguousarray(lane_feat[sl].reshape(BL * NL, D)),
            "lane_centers": np.ascontiguousarray(lane_centers[sl]),
            "x_centers": np.ascontiguousarray(x_centers[sl].reshape(BL * NA, 2)),
            "spike_rate": np.ascontiguousarray(spike_rate[sl]),
            "actor_valid": np.ascontiguousarray(avalid[sl]),
            "lane_valid": np.ascontiguousarray(lvalid[sl]),
        }
        m.update(shared)
        in_maps.append(m)
    return in_maps


def kernel(**inputs):
    global LAST_EXEC_NS
    in_maps = shard_inputs(**inputs)
    gelu_mode = os.environ.get("GELU_MODE", "hw")
    nc = _get_program(gelu_mode)
    trace = os.environ.get("BASS_KERNEL_TRACE", "0") == "1"
    res = run_bass_kernel_spmd(nc, in_maps, list(range(N_CORES)), trace=trace)
    LAST_EXEC_NS = res.exec_time_ns
    actor_out = np.concatenate(
        [r["actor_out"].reshape(BL, NA, D) for r in res.results], axis=0
    )
    lane_out = np.concatenate(
        [r["lane_out"].reshape(BL, NL, D) for r in res.results], axis=0
    )
    return actor_out, lane_out


def run_sim_core(core_inputs, gelu_mode=None, stage="full"):
    """Run CoreSim on one core's shard; returns dict of outputs."""
    from concourse.bass_interp import CoreSim

    if gelu_mode is None:
        gelu_mode = os.environ.get("GELU_MODE_SIM", "erf")
    nc = _build_program(gelu_mode, stage)
    sim = CoreSim(nc)
    for k, v in core_inputs.items():
        sim.tensor(k)[:] = v
    sim.simulate()
    return {
        "actor_out": np.array(sim.tensor("actor_out")).reshape(BL, NA, D),
        "lane_out": np.array(sim.tensor("lane_out")).reshape(BL, NL, D),
    }


# revision 20
# speedup vs baseline: 1.5374x; 1.5374x over previous
"""Trainium2 Bass kernel: EventSceneGraph GNN message passing.

Reference semantics (per scene b):
  1. top-KA of actor_valid-masked spike_rate -> aidx
  2. lane_dist[l] = min_j dist(x_centers[b, aidx[j]], lane_centers[b, l]);
     top-KL smallest over lane_valid lanes -> lidx
  3. nodes = concat(actor_feat[b, aidx], lane_feat[b, lidx])       [KA+KL, D]
  4. two residual MLP+LayerNorm blocks (rowwise over nodes)
  5. outputs = feature pools with selected rows replaced

Sharding: pure data parallel over the scene axis B=512 -> 64 scenes on each
of the 8 NeuronCores; MLP/LN params replicated.  No collectives.
"""

import os

import numpy as np

import concourse.bacc as bacc
import concourse.mybir as mybir
import concourse.tile as tile
from concourse.bass import IndirectOffsetOnAxis
from concourse.bass_utils import run_bass_kernel_spmd
from concourse.masks import make_identity

B, NA, NL, D = 512, 128, 512, 128
KA, KL = 16, 16
N_CORES = 8
BL = B // N_CORES  # scenes per core
BIG = 1e30
EPS = 1e-5

F32 = mybir.dt.float32
I32 = mybir.dt.int32
U32 = mybir.dt.uint32
AF = mybir.ActivationFunctionType
OP = mybir.AluOpType

# Abramowitz & Stegun 7.1.26 erf approximation (|err| <= 1.5e-7)
_AS_P = 0.3275911
_AS_A = (0.254829592, -0.284496736, 1.421413741, -1.453152027, 1.061405429)
_INV_SQRT2 = 0.7071067811865476

LAST_EXEC_NS = None


def _emit_gelu(nc, pool, out_sb, in_psum, mode):
    """out_sb = gelu(in_psum), exact (erf) flavor."""
    if mode == "hw":
        nc.scalar.activation(out_sb[:], in_psum[:], AF.Gelu)
        return
    # erf-composed gelu: gelu(x) = 0.5*x + 0.5*|x|*erf(|x|/sqrt(2))
    sh = list(out_sb.shape)
    ax = pool.tile(sh, F32, tag="g_ax", bufs=1)
    nc.scalar.activation(ax[:], in_psum[:], AF.Abs)
    z2 = pool.tile(sh, F32, tag="g_z2", bufs=1)
    nc.scalar.activation(z2[:], in_psum[:], AF.Square, scale=_INV_SQRT2)
    e = pool.tile(sh, F32, tag="g_e", bufs=1)
    nc.scalar.activation(e[:], z2[:], AF.Exp, scale=-1.0)
    u = pool.tile(sh, F32, tag="g_u", bufs=1)
    nc.vector.tensor_scalar(u[:], ax[:], _AS_P * _INV_SQRT2, 1.0, op0=OP.mult, op1=OP.add)
    t = pool.tile(sh, F32, tag="g_t", bufs=1)
    nc.vector.reciprocal(t[:], u[:])
    a1, a2, a3, a4, a5 = _AS_A
    h = pool.tile(sh, F32, tag="g_h", bufs=1)
    nc.vector.tensor_scalar(h[:], t[:], a5, a4, op0=OP.mult, op1=OP.add)
    for c in (a3, a2, a1):
        hm = pool.tile(sh, F32, tag="g_hm", bufs=1)
        nc.vector.tensor_tensor(hm[:], h[:], t[:], op=OP.mult)
        h = pool.tile(sh, F32, tag="g_h", bufs=1)
        nc.vector.tensor_scalar(h[:], hm[:], 1.0, c, op0=OP.mult, op1=OP.add)
    poly_t = pool.tile(sh, F32, tag="g_pt", bufs=1)
    nc.vector.tensor_tensor(poly_t[:], h[:], t[:], op=OP.mult)
    pe = pool.tile(sh, F32, tag="g_pe", bufs=1)
    nc.vector.tensor_tensor(pe[:], poly_t[:], e[:], op=OP.mult)
    erf_t = pool.tile(sh, F32, tag="g_erf", bufs=1)
    nc.vector.tensor_scalar(erf_t[:], pe[:], -1.0, 1.0, op0=OP.mult, op1=OP.add)
    xe = pool.tile(sh, F32, tag="g_xe", bufs=1)
    nc.vector.tensor_tensor(xe[:], ax[:], erf_t[:], op=OP.mult)
    halfx = pool.tile(sh, F32, tag="g_hx", bufs=1)
    nc.scalar.mul(halfx[:], in_psum[:], 0.5)
    xeh = pool.tile(sh, F32, tag="g_xeh", bufs=1)
    nc.vector.tensor_scalar(xeh[:], xe[:], 0.5, None, op0=OP.mult)
    nc.vector.tensor_tensor(out_sb[:], halfx[:], xeh[:], op=OP.add)


def _emit_gelu_bias(nc, pool, out_sb, in_psum, bias_col, mode):
    """out_sb = gelu(in_psum + bias_col), bias per-partition [D,1]."""
    if mode == "hw":
        nc.scalar.activation(out_sb[:], in_psum[:], AF.Gelu, bias=bias_col[:, 0:1])
        return
    sh = list(out_sb.shape)
    hb = pool.tile(sh, F32, tag="g_hb", name="g_hb", bufs=1)
    nc.vector.tensor_scalar(hb[:], in_psum[:], bias_col[:, 0:1], None, op0=OP.add)
    _emit_gelu(nc, pool, out_sb, hb, mode)


def _build_program(gelu_mode="hw", stage="full"):
    nc = bacc.Bacc("TRN2", target_bir_lowering=False, debug=False)

    af = nc.dram_tensor("actor_feat", [BL * NA, D], F32, kind="ExternalInput")
    lf = nc.dram_tensor("lane_feat", [BL * NL, D], F32, kind="ExternalInput")
    lc = nc.dram_tensor("lane_centers", [BL, NL, 2], F32, kind="ExternalInput")
    xc = nc.dram_tensor("x_centers", [BL * NA, 2], F32, kind="ExternalInput")
    sr = nc.dram_tensor("spike_rate", [BL, NA], F32, kind="ExternalInput")
    av = nc.dram_tensor("actor_valid", [BL, NA], F32, kind="ExternalInput")
    lv = nc.dram_tensor("lane_valid", [BL, NL], F32, kind="ExternalInput")
    wts = {
        n: nc.dram_tensor(n, [D, D], F32, kind="ExternalInput")
        for n in ("w0a", "w0b", "w1a", "w1b")
    }
    bias = {
        n: nc.dram_tensor(n, [D, 1], F32, kind="ExternalInput")
        for n in ("b0a", "b0b", "b1a", "b1b")
    }
    gam = nc.dram_tensor("gamma_col", [D, 1], F32, kind="ExternalInput")
    bet = nc.dram_tensor("beta_col", [D, 1], F32, kind="ExternalInput")

    ao = nc.dram_tensor("actor_out", [BL * NA, D], F32, kind="ExternalOutput")
    lo = nc.dram_tensor("lane_out", [BL * NL, D], F32, kind="ExternalOutput")

    do_select = stage in ("select", "passthrough", "full")
    do_mlp = stage == "full"
    do_scatter = stage in ("passthrough", "full")

    with tile.TileContext(nc) as tc:
        with (
            tc.tile_pool(name="const", bufs=1) as cp,
            tc.tile_pool(name="work", bufs=2) as wp,
            tc.tile_pool(name="mlp", bufs=2) as mp,
            tc.tile_pool(name="stats", bufs=2) as sp,
            tc.tile_pool(name="psum", bufs=2, space="PSUM") as pp,
        ):
            # ---- small input loads + constants ----
            spike_sb = cp.tile([BL, NA], F32, tag="spike")
            nc.sync.dma_start(spike_sb[:], sr[:])
            av_sb = cp.tile([BL, NA], F32, tag="av")
            nc.sync.dma_start(av_sb[:], av[:])
            lv_sb = cp.tile([BL, NL], F32, tag="lv")
            nc.sync.dma_start(lv_sb[:], lv[:])
            lc_sb = cp.tile([BL, NL, 2], F32, tag="lc")
            nc.sync.dma_start(lc_sb[:], lc[:])
            w_sb = {}
            for n, t in wts.items():
                w_sb[n] = cp.tile([D, D], F32, tag=n, name=f"{n}_sb")
                nc.sync.dma_start(w_sb[n][:], t[:])
            b_sb = {}
            for n, t in bias.items():
                b_sb[n] = cp.tile([D, 1], F32, tag=n, name=f"{n}_sb")
                nc.sync.dma_start(b_sb[n][:], t[:])
            gam_sb = cp.tile([D, 1], F32, tag="gam")
            nc.sync.dma_start(gam_sb[:], gam[:])
            bet_sb = cp.tile([D, 1], F32, tag="bet")
            nc.sync.dma_start(bet_sb[:], bet[:])

            ident = cp.tile([D, D], F32, tag="ident")
            make_identity(nc, ident[:])
            ones_row = cp.tile([1, D], F32, tag="ones_row")
            nc.vector.memset(ones_row[:], 1.0)
            ones_col = cp.tile([D, 1], F32, tag="ones_col")
            nc.vector.memset(ones_col[:], 1.0)
            base128 = cp.tile([BL, KA], I32, tag="base128")
            nc.gpsimd.iota(base128[:], pattern=[[0, KA]], base=0, channel_multiplier=NA)
            base512 = cp.tile([BL, KL], I32, tag="base512")
            nc.gpsimd.iota(base512[:], pattern=[[0, KL]], base=0, channel_multiplier=NL)

            # ---- bulk passthrough copies (DRAM->DRAM), overlap everything ----
            copy_insts_a = []
            for i in range(0, BL * NA, 4096):
                copy_insts_a.append(
                    nc.sync.dma_start(ao[i : i + 4096, :], af[i : i + 4096, :])
                )
            copy_insts_l = []
            for i in range(0, BL * NL, 4096):
                copy_insts_l.append(
                    nc.sync.dma_start(lo[i : i + 4096, :], lf[i : i + 4096, :])
                )

            afidx = None
            lfidx = None
            if do_select:
                # ---- actor top-KA over masked spike rate ----
                masked0 = wp.tile([BL, NA], F32, tag="amask0")
                nc.vector.tensor_scalar(masked0[:], av_sb[:], BIG, -BIG, op0=OP.mult, op1=OP.add)
                masked = wp.tile([BL, NA], F32, tag="amask")
                nc.vector.tensor_tensor(masked[:], masked0[:], spike_sb[:], op=OP.add)
                aidx = cp.tile([BL, KA], U32, tag="aidx")
                vals8 = wp.tile([BL, 8], F32, tag="avals8")
                nc.vector.max(vals8[:], masked[:])
                nc.vector.max_index(aidx[:, 0:8], vals8[:], masked[:])
                masked2 = wp.tile([BL, NA], F32, tag="amask2")
                nc.vector.match_replace(masked2[:], vals8[:], masked[:], -BIG)
                vals8b = wp.tile([BL, 8], F32, tag="avals8b")
                nc.vector.max(vals8b[:], masked2[:])
                nc.vector.max_index(aidx[:, 8:16], vals8b[:], masked2[:])
                aidx_i = cp.tile([BL, KA], I32, tag="aidx_i")
                nc.vector.tensor_copy(aidx_i[:], aidx[:])
                afidx = cp.tile([BL, KA], I32, tag="afidx")
                nc.vector.tensor_tensor(afidx[:], aidx_i[:], base128[:], op=OP.add)

                # ---- gather active-agent centers ----
                # HW indirect DMA consumes ONE offset per dest partition, so
                # gather slot-by-slot with [BL,1] offsets.
                ac = cp.tile([BL, KA, 2], F32, tag="ac")
                for j in range(KA):
                    nc.gpsimd.indirect_dma_start(
                        out=ac[:, j, :],
                        out_offset=None,
                        in_=xc[:],
                        in_offset=IndirectOffsetOnAxis(ap=afidx[:, j : j + 1], axis=0),
                    )

                # ---- per-lane min squared distance over the KA agents ----
                dmin = cp.tile([BL, NL], F32, tag="dmin")
                nc.vector.memset(dmin[:], BIG)
                for j in range(KA):
                    dx = wp.tile([BL, NL], F32, tag="dx")
                    nc.vector.tensor_scalar(dx[:], lc_sb[:, :, 0], ac[:, j, 0:1], None, op0=OP.subtract)
                    dy = wp.tile([BL, NL], F32, tag="dy")
                    nc.vector.tensor_scalar(dy[:], lc_sb[:, :, 1], ac[:, j, 1:2], None, op0=OP.subtract)
                    dx2 = wp.tile([BL, NL], F32, tag="dx2")
                    nc.scalar.square(dx2[:], dx[:])
                    dy2 = wp.tile([BL, NL], F32, tag="dy2")
                    nc.scalar.square(dy2[:], dy[:])
                    ss = wp.tile([BL, NL], F32, tag="ss")
                    nc.gpsimd.tensor_tensor(ss[:], dx2[:], dy2[:], op=OP.add)
                    nc.vector.tensor_tensor(dmin[:], dmin[:], ss[:], op=OP.min)

                # ---- lane top-KL (largest negated masked distance) ----
                pen = wp.tile([BL, NL], F32, tag="pen")
                nc.vector.tensor_scalar(pen[:], lv_sb[:], BIG, -BIG, op0=OP.mult, op1=OP.add)
                nld = wp.tile([BL, NL], F32, tag="nld")
                nc.vector.tensor_tensor(nld[:], pen[:], dmin[:], op=OP.subtract)
                lidx = cp.tile([BL, KL], U32, tag="lidx")
                lvals8 = wp.tile([BL, 8], F32, tag="lvals8")
                nc.vector.max(lvals8[:], nld[:])
                nc.vector.max_index(lidx[:, 0:8], lvals8[:], nld[:])
                nld2 = wp.tile([BL, NL], F32, tag="nld2")
                nc.vector.match_replace(nld2[:], lvals8[:], nld[:], -BIG)
                lvals8b = wp.tile([BL, 8], F32, tag="lvals8b")
                nc.vector.max(lvals8b[:], nld2[:])
                nc.vector.max_index(lidx[:, 8:16], lvals8b[:], nld2[:])
                lidx_i = cp.tile([BL, KL], I32, tag="lidx_i")
                nc.vector.tensor_copy(lidx_i[:], lidx[:])
                lfidx = cp.tile([BL, KL], I32, tag="lfidx")
                nc.vector.tensor_tensor(lfidx[:], lidx_i[:], base512[:], op=OP.add)

                # ---- offset re-layout [BL,K] -> [128, BL/8] (one offset per
                # dest partition per batch of 128 rows = 8 scenes x 16 slots)
                with tc.tile_pool(name="dram", bufs=1, space="DRAM") as dp:
                    stg_a = dp.tile([BL, KA], I32, tag="stg_a")
                    nc.sync.dma_start(stg_a[:], afidx[:])
                    ofs_a = cp.tile([128, BL // 8], I32, tag="ofs_a")
                    nc.sync.dma_start(
                        ofs_a[:], stg_a[:].rearrange("(g sg) j -> sg j g", g=BL // 8)
                    )
                    stg_l = dp.tile([BL, KL], I32, tag="stg_l")
                    nc.sync.dma_start(stg_l[:], lfidx[:])
                    ofs_l = cp.tile([128, BL // 8], I32, tag="ofs_l")
                    nc.sync.dma_start(
                        ofs_l[:], stg_l[:].rearrange("(g sg) j -> sg j g", g=BL // 8)
                    )

            if do_select and not do_mlp:
                # passthrough debug stage: gather each 128-row batch, scatter
                # it straight back
                for ci in range(2 * (BL // 8)):
                    if ci < BL // 8:
                        feat, outt, ofs, g, cdeps = af, ao, ofs_a, ci, copy_insts_a
                    else:
                        feat, outt, ofs, g, cdeps = lf, lo, ofs_l, ci - BL // 8, copy_insts_l
                    x = mp.tile([D, D], F32, tag="x0", name="x_pt")
                    nc.gpsimd.indirect_dma_start(
                        out=x[:], out_offset=None, in_=feat[:],
                        in_offset=IndirectOffsetOnAxis(ap=ofs[:, g : g + 1], axis=0),
                    )
                    if do_scatter:
                        sc = nc.gpsimd.indirect_dma_start(
                            out=outt[:],
                            out_offset=IndirectOffsetOnAxis(ap=ofs[:, g : g + 1], axis=0),
                            in_=x[:], in_offset=None,
                        )
                        for c in cdeps:
                            tile.add_dep_helper(sc.ins, c.ins, reason="scatter after bulk copy")

            if do_select and do_mlp:
                # ---- MLP in transposed layout over superchunks of 512 rows.
                # All tensors live as [D, rows]: weights go in as lhsT
                # directly, biases/gamma/beta are per-partition scalars, LN
                # stats come from PE ones-matvecs, per-row scale/shift is
                # replicated across partitions with K=1 ones matmuls.
                layers_T = [
                    (w_sb["w0a"], b_sb["b0a"], w_sb["w0b"], b_sb["b0b"]),
                    (w_sb["w1a"], b_sb["b1a"], w_sb["w1b"], b_sb["b1b"]),
                ]
                SCB = 4  # 128-row gather batches per superchunk
                R = SCB * D  # 512 rows
                n_sc_pool = (BL // 8) // SCB  # superchunks per feature pool
                for sc_i in range(2 * n_sc_pool):
                    if sc_i < n_sc_pool:
                        feat, outt, ofs, gb, cdeps = af, ao, ofs_a, sc_i * SCB, copy_insts_a
                    else:
                        feat, outt, ofs, gb, cdeps = (
                            lf, lo, ofs_l, (sc_i - n_sc_pool) * SCB, copy_insts_l)
                    # gather 4x128 rows, transpose into xT [D, R]
                    tq = pp.tile([D, R], F32, tag="tq", bufs=2)
                    for j in range(SCB):
                        xg = mp.tile([D, D], F32, tag="xg")
                        nc.gpsimd.indirect_dma_start(
                            out=xg[:], out_offset=None, in_=feat[:],
                            in_offset=IndirectOffsetOnAxis(ap=ofs[:, gb + j : gb + j + 1], axis=0),
                        )
                        nc.tensor.transpose(tq[:, j * D : (j + 1) * D], xg[:], ident[:])
                    xT = mp.tile([D, R], F32, tag="xT", name="xT_in")
                    nc.vector.tensor_copy(xT[:], tq[:])
                    for (wa, ba_c, wb, bb_c) in layers_T:
                        h1 = pp.tile([D, R], F32, tag="h1", bufs=1)
                        nc.tensor.matmul(h1[:], lhsT=wa[:], rhs=xT[:], start=True, stop=True)
                        gT = mp.tile([D, R], F32, tag="gT")
                        _emit_gelu_bias(nc, mp, gT, h1, ba_c, gelu_mode)
                        h2 = pp.tile([D, R], F32, tag="h2", bufs=1)
                        nc.tensor.matmul(h2[:], lhsT=wb[:], rhs=gT[:], start=True, stop=True)
                        tmpb = mp.tile([D, R], F32, tag="tmpb")
                        nc.vector.tensor_scalar(tmpb[:], h2[:], bb_c[:, 0:1], None, op0=OP.add)
                        x1 = mp.tile([D, R], F32, tag="x1")
                        nc.vector.tensor_tensor(x1[:], tmpb[:], xT[:], op=OP.add)
                        sq = mp.tile([D, R], F32, tag="sq")
                        nc.gpsimd.tensor_tensor(sq[:], x1[:], x1[:], op=OP.mult)
                        sums_p = pp.tile([1, R], F32, tag="st1", bufs=1)
                        nc.tensor.matmul(sums_p[:], lhsT=ones_col[:], rhs=x1[:], start=True, stop=True)
                        sumsq_p = pp.tile([1, R], F32, tag="st2", bufs=1)
                        nc.tensor.matmul(sumsq_p[:], lhsT=ones_col[:], rhs=sq[:], start=True, stop=True)
                        negm = sp.tile([1, R], F32, tag="negm")
                        nc.vector.tensor_scalar(negm[:], sums_p[:], -1.0 / D, None, op0=OP.mult)
                        msq = sp.tile([1, R], F32, tag="msq")
                        nc.vector.tensor_tensor(msq[:], negm[:], negm[:], op=OP.mult)
                        v1 = sp.tile([1, R], F32, tag="v1")
                        nc.vector.tensor_scalar(v1[:], sumsq_p[:], 1.0 / D, EPS, op0=OP.mult, op1=OP.add)
                        v2 = sp.tile([1, R], F32, tag="v2")
                        nc.vector.tensor_tensor(v2[:], v1[:], msq[:], op=OP.subtract)
                        sd = sp.tile([1, R], F32, tag="sd")
                        nc.scalar.sqrt(sd[:], v2[:])
                        rstd = sp.tile([1, R], F32, tag="rstd")
                        nc.vector.reciprocal(rstd[:], sd[:])
                        negmr = sp.tile([1, R], F32, tag="negmr")
                        nc.vector.tensor_tensor(negmr[:], negm[:], rstd[:], op=OP.mult)
                        rr = pp.tile([D, R], F32, tag="rr", bufs=1)
                        nc.tensor.matmul(rr[:], lhsT=ones_row[:], rhs=rstd[:], start=True, stop=True)
                        mr = pp.tile([D, R], F32, tag="mr", bufs=1)
                        nc.tensor.matmul(mr[:], lhsT=ones_row[:], rhs=negmr[:], start=True, stop=True)
                        t1 = mp.tile([D, R], F32, tag="t1")
                        nc.vector.tensor_tensor(t1[:], x1[:], rr[:], op=OP.mult)
                        t2 = mp.tile([D, R], F32, tag="t2")
                        nc.vector.tensor_tensor(t2[:], t1[:], mr[:], op=OP.add)
                        xT = mp.tile([D, R], F32, tag="xT", name="xT_out")
                        nc.vector.tensor_scalar(xT[:], t2[:], gam_sb[:, 0:1], bet_sb[:, 0:1], op0=OP.mult, op1=OP.add)
                    # transpose back to row-major chunks + scatter
                    for j in range(SCB):
                        ft = pp.tile([D, D], F32, tag="tq", bufs=2)
                        nc.tensor.transpose(ft[:], xT[:, j * D : (j + 1) * D], ident[:])
                        xo = mp.tile([D, D], F32, tag="xo")
                        nc.vector.tensor_copy(xo[:], ft[:])
                        if do_scatter:
                            sc = nc.gpsimd.indirect_dma_start(
                                out=outt[:],
                                out_offset=IndirectOffsetOnAxis(ap=ofs[:, gb + j : gb + j + 1], axis=0),
                                in_=xo[:], in_offset=None,
                            )
                            for c in cdeps:
                                tile.add_dep_helper(sc.ins, c.ins, reason="scatter after bulk copy")

    nc.compile()
    return nc


_PROGRAMS = {}


def _get_program(gelu_mode, stage="full"):
    key = (gelu_mode, stage)
    if key not in _PROGRAMS:
        _PROGRAMS[key] = _build_program(gelu_mode, stage)
    return _PROGRAMS[key]


def shard_inputs(actor_feat, lane_feat, lane_centers, x_centers, spike_rate,
                 actor_valid, lane_valid,
                 W0a, b0a, W0b, b0b, W1a, b1a, W1b, b1b, gamma, beta):
    f32 = lambda a: np.ascontiguousarray(np.asarray(a), dtype=np.float32)
    actor_feat = f32(actor_feat)
    lane_feat = f32(lane_feat)
    lane_centers = f32(lane_centers)
    x_centers = f32(x_centers)
    spike_rate = f32(spike_rate)
    avalid = np.asarray(actor_valid).astype(np.float32)
    lvalid = np.asarray(lane_valid).astype(np.float32)
    shared = {
        "w0a": f32(W0a), "w0b": f32(W0b), "w1a": f32(W1a), "w1b": f32(W1b),
        "b0a": f32(b0a).reshape(D, 1), "b0b": f32(b0b).reshape(D, 1),
        "b1a": f32(b1a).reshape(D, 1), "b1b": f32(b1b).reshape(D, 1),
        "gamma_col": f32(gamma).reshape(D, 1),
        "beta_col": f32(beta).reshape(D, 1),
    }
    in_maps = []
    for c in range(N_CORES):
        sl = slice(c * BL, (c + 1) * BL)
        m = {
            "actor_feat": np.ascontiguousarray(actor_feat[sl].reshape(BL * NA, D)),
            "lane_feat": np.ascontiguousarray(lane_feat[sl].reshape(BL * NL, D)),
            "lane_centers": np.ascontiguousarray(lane_centers[sl]),
            "x_centers": np.ascontiguousarray(x_centers[sl].reshape(BL * NA, 2)),
            "spike_rate": np.ascontiguousarray(spike_rate[sl]),
            "actor_valid": np.ascontiguousarray(avalid[sl]),
            "lane_valid": np.ascontiguousarray(lvalid[sl]),
        }
        m.update(shared)
        in_maps.append(m)
    return in_maps


def kernel(**inputs):
    global LAST_EXEC_NS
    in_maps = shard_inputs(**inputs)
    gelu_mode = os.environ.get("GELU_MODE", "hw")
    nc = _get_program(gelu_mode)
    trace = os.environ.get("BASS_KERNEL_TRACE", "0") == "1"
    res = run_bass_kernel_spmd(nc, in_maps, list(range(N_CORES)), trace=trace)
    LAST_EXEC_NS = res.exec_time_ns
    actor_out = np.concatenate(
        [r["actor_out"].reshape(BL, NA, D) for r in res.results], axis=0
    )
    lane_out = np.concatenate(
        [r["lane_out"].reshape(BL, NL, D) for r in res.results], axis=0
    )
    return actor_out, lane_out


def run_sim_core(core_inputs, gelu_mode=None, stage="full"):
    """Run CoreSim on one core's shard; returns dict of outputs."""
    from concourse.bass_interp import CoreSim

    if gelu_mode is None:
        gelu_mode = os.environ.get("GELU_MODE_SIM", "erf")
    nc = _build_program(gelu_mode, stage)
    sim = CoreSim(nc)
    for k, v in core_inputs.items():
        sim.tensor(k)[:] = v
    sim.simulate()
    return {
        "actor_out": np.array(sim.tensor("actor_out")).reshape(BL, NA, D),
        "lane_out": np.array(sim.tensor("lane_out")).reshape(BL, NL, D),
    }
